# revision 13
# baseline (speedup 1.0000x reference)
"""Trainium2 Bass kernel for nn_DetectionLoss (2-class detection loss).

Computes, over B=2^24 rows of logits [B,2] and labels [B]:
  ce    = mean(-log_softmax(outputs)[label])
  pred  = argmax(outputs, axis=1)
  confusion counts TP/TN/FP/FN from (label, pred)
  CS    = M[pred, label] with M = [[0,1],[0,0]]  -> mean(CS) = FN/B
  loss  = ce + coeff(TP,TN,FP,FN) * mean(CS)

Key identity used on device (2 classes): with d = x1 - x0, h = label - 0.5
and sgn = 1 - 2*label = -2*h in {+1,-1},
  ce_row = softplus(sgn * d) = log(1 + exp(-2*h*d))
and the confusion counts follow from three streaming sums:
  H1 = sum(h), C1 = sum(pred*h), D1 = sum(pred + h)
The device streams the inputs once, producing partial sums per
(core, chunk, partition); the tiny [ncores, nchunks, 128, 4] tensor is
combined on the host (exact fp64 half-integer arithmetic for counts).

Sharding: data-parallel over the batch dim across 8 NeuronCores.
"""

import numpy as np

import concourse.bass as bass
import concourse.mybir as mybir
import concourse.tile as tile
from concourse.bass_utils import run_bass_kernel_spmd

N_CORES = 8
P = 128
LAMBD = 0.5
NCHUNKS = 8

_cache = {}

_MAX_WAITS = 1  # this walrus build rejects >1 embedded sync-wait per instruction


def _split_multiwaits(nc):
    """Walrus in this container can't encode instructions with multiple
    sync waits; hoist all but the last into standalone EventSemaphore
    waits on the same engine immediately before the instruction."""
    n = [0]

    def fix_block(blk):
        new_insts = []
        for ins in blk.instructions:
            si = ins.sync_info
            if si is not None and si.on_wait and len(si.on_wait) > _MAX_WAITS:
                waits = list(si.on_wait)
                for w in waits[: -_MAX_WAITS]:
                    n[0] += 1
                    ev = mybir.InstEventSemaphore(
                        name=f"I-waitsplit-{n[0]}",
                        ins=[],
                        outs=[],
                        sync_info=mybir.SyncInfo(on_wait=[w], on_update=[]),
                    )
                    ev.engine = ins.engine
                    new_insts.append(ev)
                si.on_wait = waits[-_MAX_WAITS:]
            new_insts.append(ins)
        blk.instructions = new_insts

    for fn in nc.m.functions:
        for blk in fn.blocks:
            fix_block(blk)


def _build(rows_per_core: int, nchunks: int, lab64: bool):
    """Build the per-core Bass module. All cores run the same program on
    their own shard (pure data parallel, no collectives)."""
    key = (rows_per_core, nchunks, lab64)
    if key in _cache:
        return _cache[key]

    F = rows_per_core // (P * nchunks)  # rows per partition per chunk
    assert rows_per_core == F * P * nchunks, (rows_per_core, nchunks)

    nc = bass.Bass(trn_type="TRN2")
    dtf = mybir.dt.float32
    dti = mybir.dt.int32
    dtb = mybir.dt.bfloat16
    Op = mybir.AluOpType
    Act = mybir.ActivationFunctionType

    LW = 2 if lab64 else 1  # int32 words per label
    x = nc.dram_tensor("x", [nchunks, P, 2 * F], dtf, kind="ExternalInput")
    lab = nc.dram_tensor("lab", [nchunks, P, LW * F], dti, kind="ExternalInput")
    acc = nc.dram_tensor("acc", [nchunks, P, 4], dtf, kind="ExternalOutput")

    with tile.TileContext(nc) as tc:
        with (
            tc.tile_pool(name="io", bufs=3) as io_pool,
            tc.tile_pool(name="mid", bufs=2) as mid,
            tc.tile_pool(name="junk", bufs=2) as junk,
            tc.tile_pool(name="st", bufs=3) as stp,
        ):
            for c in range(nchunks):
                xt = io_pool.tile([P, 2 * F], dtf, tag="xt")
                nc.sync.dma_start(out=xt, in_=x[c])
                lt = io_pool.tile([P, LW * F], dti, tag="lt")
                nc.sync.dma_start(out=lt, in_=lab[c])
                st = stp.tile([P, 4], dtf, tag="st")

                xp = xt.rearrange("p (f two) -> p f two", two=2)
                if lab64:
                    # int64 labels arrive as little-endian int32 pairs; the
                    # low word (stride 2) holds the value.
                    lv = lt.rearrange("p (f two) -> p f two", two=2)[:, :, 0]
                else:
                    lv = lt[:]

                # h = label - 0.5 in {-0.5,+0.5}. (The accum-bearing
                # TensorScalarCacheReduce encoding rejects int inputs, so
                # the cast pass carries no accum; H1 = sum(h) comes from a
                # cheap single-src bf16 pass right after.)
                h = mid.tile([P, F], dtb, tag="h")
                nc.vector.tensor_scalar(
                    out=h,
                    in0=lv,
                    scalar1=0.5,
                    scalar2=None,
                    op0=Op.subtract,
                )
                j0 = junk.tile([P, F], dtb, tag="j0")
                nc.vector.tensor_scalar(
                    out=j0,
                    in0=h,
                    scalar1=0.0,
                    scalar2=None,
                    op0=Op.add,
                    op1=Op.add,
                    accum_out=st[:, 0:1],
                )

                # d = x1 - x0
                d = mid.tile([P, F], dtb, tag="d")
                nc.vector.tensor_sub(out=d, in0=xp[:, :, 1], in1=xp[:, :, 0])

                # pred = (d > 0); accum C1 = sum(pred*h), D1 = sum(pred+h)
                j1 = junk.tile([P, F], dtb, tag="j1")
                nc.vector.scalar_tensor_tensor(
                    out=j1,
                    in0=d,
                    scalar=0.0,
                    in1=h,
                    op0=Op.is_gt,
                    op1=Op.mult,
                    accum_out=st[:, 1:2],
                )
                j2 = junk.tile([P, F], dtb, tag="j2")
                nc.vector.scalar_tensor_tensor(
                    out=j2,
                    in0=d,
                    scalar=0.0,
                    in1=h,
                    op0=Op.is_gt,
                    op1=Op.add,
                    accum_out=st[:, 2:3],
                )

                # u = d*h; ce_row = log(1 + exp(-2*u)) with the -2 folded
                # into the Exp activation's scale.
                u = mid.tile([P, F], dtb, tag="u")
                nc.vector.tensor_mul(out=u, in0=d, in1=h)
                t = mid.tile([P, F], dtb, tag="t")
                nc.scalar.activation(out=t, in_=u, func=Act.Exp, scale=-2.0)
                j3 = junk.tile([P, F], dtf, tag="j3")
                nc.scalar.activation(
                    out=j3,
                    in_=t,
                    func=Act.Ln,
                    bias=1.0,
                    scale=1.0,
                    accum_out=st[:, 3:4],
                )

                nc.sync.dma_start(out=acc[c], in_=st)

    _cache[key] = (nc, F)
    return nc, F


def _combine(acc: np.ndarray, B: int) -> np.ndarray:
    """Host-side scalar epilogue from the stacked per-core accumulators.

    acc: [n_cores, nchunks, P, 4] float32. Counts are exact half-integers
    in fp32 at every accumulation stage; fp64 here keeps them exact."""
    a = acc.astype(np.float64).sum(axis=(0, 1, 2))
    H1, C1, D1, CE = a
    n1 = H1 + B / 2.0  # labels == 1
    p1 = D1 - H1  # preds == 1
    TP = C1 + p1 / 2.0
    FP = p1 - TP
    FN = n1 - TP
    TN = B - n1 - p1 + TP

    ce = CE / B
    mean_cs = FN / B
    nonzero = (TP > 0) and (TN > 0) and (FP > 0) and (FN > 0)
    ratio = (TP / max(TP + FN, 1.0)) * (FP / max(FP + TN, 1.0))
    if nonzero:
        coeff = -LAMBD * np.log(np.sqrt(max(ratio, 1e-30)))
    else:
        coeff = LAMBD
    return np.array(ce + coeff * mean_cs, dtype=np.float32)


def run(outputs: np.ndarray, labels: np.ndarray):
    """Run on 8 cores; returns (loss, BassKernelResults)."""
    outputs = np.asarray(outputs)
    labels = np.asarray(labels)
    B = outputs.shape[0]
    assert outputs.shape == (B, 2) and labels.shape == (B,)
    assert B % N_CORES == 0
    S = B // N_CORES

    lab64 = labels.dtype.itemsize == 8
    nc, F = _build(S, NCHUNKS, lab64)
    _split_multiwaits(nc)  # idempotent; CoreSim needs the unsplit module
    LW = 2 if lab64 else 1

    in_maps = []
    for i in range(N_CORES):
        xs = np.ascontiguousarray(outputs[i * S : (i + 1) * S], dtype=np.float32)
        xs = xs.reshape(NCHUNKS, P, 2 * F)
        ls = np.ascontiguousarray(labels[i * S : (i + 1) * S])
        ls = ls.view(np.int32).reshape(NCHUNKS, P, LW * F)
        in_maps.append({"x": xs, "lab": ls})

    res = run_bass_kernel_spmd(nc, in_maps, core_ids=list(range(N_CORES)))
    acc = np.stack([r["acc"] for r in res.results])
    return _combine(acc, B), res


def kernel(outputs: np.ndarray, labels: np.ndarray) -> np.ndarray:
    return run(outputs, labels)[0]
